# revision 25
# baseline (speedup 1.0000x reference)
"""Trainium2 Bass kernel for nn_MoEAugmentedActor (moe_routing).

Strategy (pure data parallel, 8 cores, batch-sharded):
  - Host prepares a feature-major fp16 view xT of the needed x columns
    (579 of 975), with ones-rows baked in so L1 biases ride the matmul.
  - On-chip everything is feature-major: [features(part), batch(free)],
    batch tiled at 512 columns.
  - ELU(y) is computed as  elu(y)+1 = max(y+1, min(e^y, 1)):
      psum holds y+1 (bias rows are b+1), ACT does t = Exp(psum-1),
      DVE scalar_tensor_tensor does u = (t min 1) max psum  in one pass.
    The +1 shift is absorbed into the next layer's bias on the host
    (b' = b - colsum(W)).
  - Gate logits are replicated into 32-aligned 29-row blocks by an
    expanded gate-L2 matmul so per-expert softmax weights can be read
    as legal SBUF slices; softmax runs without max-subtraction
    (logits are tiny).  Blend: se_e = (pacts_e + b3'_e) * exp(gl_e)
    via one scalar_tensor_tensor per expert, summed, then multiplied
    by the broadcast reciprocal of the partition-summed exp.
  - Device writes out feature-major [29, n]; host transposes back.
"""

import os
import sys

for _p in ("/opt/trn_rl_repo", "/root/.axon_site/_ro/trn_rl_repo"):
    if os.path.isdir(_p) and _p not in sys.path:
        sys.path.insert(0, _p)

import numpy as np

# ----------------------------------------------------------------- constants
N_FULL = 131072
N_CORES = 8
N_CORE = N_FULL // N_CORES  # 16384
TILE = 512  # batch columns per tile

OBS_TERM_DIMS = (3, 3, 3, 3, 29, 29, 29, 96)
HISTORY_LEN = 5
_OFFS = [0]
for _d in OBS_TERM_DIMS[:-1]:
    _OFFS.append(_OFFS[-1] + _d * HISTORY_LEN)

# vae_hist column order: frame i in 0..4, terms 1..6, dims within term
VAE_COLS = [
    _OFFS[t] + i * OBS_TERM_DIMS[t] + j
    for i in range(HISTORY_LEN)
    for t in range(1, 7)
    for j in range(OBS_TERM_DIMS[t])
]  # 480
OT_COLS = [
    _OFFS[t] + 4 * OBS_TERM_DIMS[t] + j for t in range(7) for j in range(OBS_TERM_DIMS[t])
]  # 99
ELEV_COLS = list(range(_OFFS[7] + 4 * 96, _OFFS[7] + 5 * 96))  # 96

XT_ROWS = 784  # 6 blocks of 128 + zeros/ones tail
WCOLS = 4224


def _w_offsets():
    off = {}
    c = 0

    def take(name, n):
        nonlocal c
        off[name] = c
        c += n

    take("w1", 4 * 256)       # 4 chunks x [K,256]
    take("wzv", 2 * 35)       # 2 chunks x [128,35]  ([Wv|Wz] order)
    take("ae1", 64)           # [97,64]
    take("ae2", 32)           # [64,32]
    take("g1", 64)            # [33,64] stored at partitions 64..96
    take("g2", 5)             # [64,5]
    take("g2r1", 128)         # [64,128] replicated gate cols, experts 0..3
    take("g2r2", 29)          # [64,29]  replicated gate col, expert 4
    take("e1a", 5 * 128)      # [99,128] x5
    take("e1b", 5 * 128)      # [97,128] x5
    take("c2", 5 * 128)       # [1,128] x5
    take("e2", 5 * 128)       # [128,128] x5
    take("e3", 5 * 32)        # [128,32] x5 (padded to 32)
    take("ones5", 1)          # [5,1]
    take("msum", 29)          # [128,29] 0/1 block-sum matrix
    take("i29", 29)           # [29,29] identity
    assert c <= WCOLS, c
    return off


WOFF = _w_offsets()

# bpack columns
BC_ZV = 0      # rows 0..34:  [bv|bz]' adjusted
BC_ZE = 1      # rows 0..31:  ae_b2'
BC_G2 = 2      # rows 0..4:   gate_b2'
BC_NEG1 = 3    # all rows: -1.0
BC_G2R = 4     # rows 32e+k (e<4,k<29): gate_b2'_e   (replicated-logit bias)
BC_G2R4 = 5    # rows 0..28: gate_b2'_4
BC_B3 = 6      # rows 32e+k (e<4,k<29): b3'_e[k]
BC_B34 = 7     # rows 0..28: b3'_4
BC_C2 = 8      # cols 8..12:  expert-L2 bias c2_e (rows 0..127)
BC_C2P1 = 13   # cols 13..17: c2_e + 1
NBCOLS = 18


# ----------------------------------------------------------------- device IR

def build_program(n_rows=N_CORE, num_devices=N_CORES):
    """Build + compile the per-core Bass program. Returns nc."""
    import concourse.bass as bass
    import concourse.mybir as mybir
    from concourse import bacc
    from concourse.tile import TileContext

    fp16 = mybir.dt.float16
    fp32 = mybir.dt.float32
    AF = mybir.ActivationFunctionType
    OP = mybir.AluOpType

    n_tiles = n_rows // TILE
    assert n_rows % TILE == 0

    nc = bacc.Bacc("TRN2", target_bir_lowering=False, debug=False,
                   num_devices=num_devices)

    xT = nc.dram_tensor("xT", (XT_ROWS, n_rows), fp16, kind="ExternalInput").ap()
    wpack = nc.dram_tensor("wpack", (128, WCOLS), fp16, kind="ExternalInput").ap()
    bpack = nc.dram_tensor("bpack", (128, NBCOLS), fp32, kind="ExternalInput").ap()
    out_fm = nc.dram_tensor("out_fm", (29, n_rows), fp32, kind="ExternalOutput").ap()

    with TileContext(nc) as tc:
        with (
            tc.tile_pool(name="const", bufs=1) as constp,
            tc.tile_pool(name="xio", bufs=4) as xio,
            tc.tile_pool(name="inp", bufs=4) as inpp,
            tc.tile_pool(name="uh", bufs=6) as uhp,
            tc.tile_pool(name="small", bufs=3) as smallp,
            tc.tile_pool(name="texp", bufs=10) as texpp,
            tc.tile_pool(name="u12", bufs=8) as u12p,
            tc.tile_pool(name="blend", bufs=4) as blendp,
            tc.tile_pool(name="pe", bufs=2, space="PSUM") as pep,
            tc.tile_pool(name="pmain", bufs=3, space="PSUM") as pmainp,
            tc.tile_pool(name="pacts", bufs=1, space="PSUM") as pactsp,
        ):
            # ---- persistent constants
            wsb = constp.tile([128, WCOLS], fp16, tag="wsb")
            nc.sync.dma_start(out=wsb, in_=wpack)
            bsb = constp.tile([128, NBCOLS], fp32, tag="bsb")
            nc.sync.dma_start(out=bsb, in_=bpack)
            ones_all = constp.tile([128, TILE], fp16, tag="ones_all")
            nc.vector.memset(ones_all, 1.0)

            xT_blk = xT[0:640].rearrange("(b p) n -> p b n", p=128)  # [128, 5, n]

            def w(name, k, m, idx=0, msz=None, prow=0):
                base = WOFF[name] + idx * m
                return wsb[prow:prow + k, base:base + (msz if msz is not None else m)]

            def elu(pool, tag, psum, m, fd=TILE):
                """psum[0:m, 0:fd] holds y+1 -> u = elu(y)+1 fp16."""
                t = texpp.tile([128, 2 * TILE], fp16, tag="texp")
                nc.scalar.activation(t[0:m, 0:fd], psum[0:m, 0:fd], AF.Exp,
                                     bias=bsb[0:m, BC_NEG1:BC_NEG1 + 1], scale=1.0)
                u = pool.tile([128, 2 * TILE], fp16, tag=tag)
                nc.vector.scalar_tensor_tensor(
                    out=u[0:m, 0:fd], in0=t[0:m, 0:fd], scalar=1.0,
                    in1=psum[0:m, 0:fd], op0=OP.min, op1=OP.max)
                return u

            for it in range(n_tiles):
                n0 = it * TILE
                # ---- loads
                xsb = xio.tile([128, 5, TILE], fp16, tag="xsb")
                nc.sync.dma_start(out=xsb, in_=xT_blk[:, 0:5, n0:n0 + TILE])
                inpA = inpp.tile([128, TILE], fp16, tag="inpA")
                nc.sync.dma_start(out=inpA[0:99], in_=xT[640:739, n0:n0 + TILE])
                inpB = inpp.tile([97, TILE], fp16, tag="inpB")
                nc.sync.dma_start(out=inpB[32:64], in_=xT[739:771, n0:n0 + TILE])
                nc.sync.dma_start(out=inpB[96:97], in_=xT[771:772, n0:n0 + TILE])

                # ---- AE: ha -> z_E(32) into inpB[64:96]
                pa = pmainp.tile([128, TILE], fp32, tag="pmain")
                nc.tensor.matmul(pa[0:64], lhsT=w("ae1", 97, 64), rhs=xsb[0:97, 4, :],
                                 start=True, stop=True)
                u_a = elu(uhp, "uh", pa, 64)
                pzE = pmainp.tile([128, TILE], fp32, tag="pmain")
                nc.tensor.matmul(pzE[0:32], lhsT=w("ae2", 64, 32), rhs=u_a[0:64, 0:TILE],
                                 start=True, stop=True)
                nc.scalar.activation(inpB[64:96], pzE[0:32], AF.Identity,
                                     bias=bsb[0:32, BC_ZE:BC_ZE + 1], scale=1.0)

                # ---- VAE L1: two 128-halves, separate psum tiles
                u_hs = []
                for half in (0, 1):
                    ph = pmainp.tile([128, TILE], fp32, tag="pmain")
                    for c in range(4):
                        k = 121 if c == 3 else 120
                        nc.tensor.matmul(
                            ph,
                            lhsT=wsb[0:k, WOFF["w1"] + c * 256 + half * 128:
                                     WOFF["w1"] + c * 256 + half * 128 + 128],
                            rhs=xsb[0:k, c, :],
                            start=(c == 0), stop=(c == 3))
                    u_hs.append(elu(uhp, "uh", ph, 128))
                u_h0, u_h1 = u_hs

                # ---- VAE L2 -> [v_pred(3) | z_H(32)] into inpB[0:35]
                pz = pmainp.tile([128, TILE], fp32, tag="pmain")
                nc.tensor.matmul(pz[0:35], lhsT=w("wzv", 128, 35, 0),
                                 rhs=u_h0[:, 0:TILE], start=True, stop=False)
                nc.tensor.matmul(pz[0:35], lhsT=w("wzv", 128, 35, 1),
                                 rhs=u_h1[:, 0:TILE], start=False, stop=True)
                nc.scalar.activation(inpB[0:35], pz[0:35], AF.Identity,
                                     bias=bsb[0:35, BC_ZV:BC_ZV + 1], scale=1.0)

                # ---- experts: pairs (0,1)+(2,3) interleaved; e4 single lane
                pacts0 = pactsp.tile([128, TILE], fp32, tag="pacts")

                def l1_mms(pair):
                    pe1 = pep.tile([128, 2 * TILE], fp32, tag="pe")
                    for j, e in enumerate(pair):
                        sl = slice(j * TILE, (j + 1) * TILE)
                        nc.tensor.matmul(pe1[:, sl], lhsT=w("e1a", 99, 128, e),
                                         rhs=inpA[0:99], start=True, stop=False)
                        nc.tensor.matmul(pe1[:, sl], lhsT=w("e1b", 97, 128, e),
                                         rhs=inpB[0:97], start=False, stop=True)
                    return pe1

                def l2_mms(pair, u1):
                    pe2 = pep.tile([128, 2 * TILE], fp32, tag="pe")
                    for j, e in enumerate(pair):
                        sl = slice(j * TILE, (j + 1) * TILE)
                        nc.tensor.matmul(pe2[:, sl], lhsT=w("e2", 128, 128, e),
                                         rhs=u1[:, sl], start=True, stop=True)
                    return pe2

                def l2_elu(pair, pe2):
                    fd = len(pair) * TILE
                    t2 = texpp.tile([128, 2 * TILE], fp16, tag="texp")
                    for j, e in enumerate(pair):
                        sl = slice(j * TILE, (j + 1) * TILE)
                        nc.scalar.activation(t2[:, sl], pe2[:, sl], AF.Exp,
                                             bias=bsb[0:128, BC_C2 + e:BC_C2 + e + 1],
                                             scale=1.0)
                    s2 = texpp.tile([128, 2 * TILE], fp16, tag="s2")
                    nc.vector.tensor_scalar(out=s2[:, 0:fd], in0=t2[:, 0:fd],
                                            scalar1=1.0, scalar2=None, op0=OP.min)
                    u2 = u12p.tile([128, 2 * TILE], fp16, tag="u12")
                    for j, e in enumerate(pair):
                        sl = slice(j * TILE, (j + 1) * TILE)
                        nc.vector.scalar_tensor_tensor(
                            out=u2[:, sl], in0=pe2[:, sl],
                            scalar=bsb[0:128, BC_C2P1 + e:BC_C2P1 + e + 1],
                            in1=s2[:, sl], op0=OP.add, op1=OP.max)
                    return u2

                def l3_mms(pair, u2):
                    for j, e in enumerate(pair):
                        sl = slice(j * TILE, (j + 1) * TILE)
                        if e < 4:
                            nc.tensor.matmul(pacts0[32 * e:32 * e + 32],
                                             lhsT=w("e3", 128, 32, e), rhs=u2[:, sl],
                                             start=True, stop=True,
                                             tile_position=(0, 32 * e))
                        else:
                            pacts1 = pmainp.tile([128, TILE], fp32, tag="pmain")
                            nc.tensor.matmul(pacts1[0:29],
                                             lhsT=w("e3", 128, 32, e, msz=29),
                                             rhs=u2[:, sl], start=True, stop=True)
                            globals_pacts1[0] = pacts1
                    return globals_pacts1[0] if pair == (4,) else None

                globals_pacts1 = [None]
                pA, pB = (0, 1), (2, 3)
                pe1a = l1_mms(pA)
                pe1b = l1_mms(pB)
                u1a = elu(u12p, "u12", pe1a, 128, 2 * TILE)
                pe2a = l2_mms(pA, u1a)
                u1b = elu(u12p, "u12", pe1b, 128, 2 * TILE)
                pe2b = l2_mms(pB, u1b)
                u2a = l2_elu(pA, pe2a)
                l3_mms(pA, u2a)
                u2b = l2_elu(pB, pe2b)
                l3_mms(pB, u2b)
                # expert 4 single lane
                pe14 = pmainp.tile([128, TILE], fp32, tag="pmain")
                nc.tensor.matmul(pe14, lhsT=w("e1a", 99, 128, 4),
                                 rhs=inpA[0:99], start=True, stop=False)
                nc.tensor.matmul(pe14, lhsT=w("e1b", 97, 128, 4),
                                 rhs=inpB[0:97], start=False, stop=True)
                u14 = elu(u12p, "u12", pe14, 128)
                pe24 = pmainp.tile([128, TILE], fp32, tag="pmain")
                nc.tensor.matmul(pe24, lhsT=w("e2", 128, 128, 4),
                                 rhs=u14[:, 0:TILE], start=True, stop=True)
                u24 = l2_elu((4,), pe24)
                pacts1 = l3_mms((4,), u24)

                # ---- gate chain (emitted late: overlaps expert crunch)
                pg = pmainp.tile([128, TILE], fp32, tag="pmain")
                nc.tensor.matmul(pg[0:64], lhsT=w("g1", 33, 64, prow=64),
                                 rhs=inpB[64:97], start=True, stop=True)
                u_g = elu(uhp, "uh", pg, 64)
                pgl = pmainp.tile([128, TILE], fp32, tag="pmain")
                nc.tensor.matmul(pgl[0:5], lhsT=w("g2", 64, 5), rhs=u_g[0:64, 0:TILE],
                                 start=True, stop=True)
                t_gate = smallp.tile([5, TILE], fp16, tag="tgate")
                nc.scalar.activation(t_gate, pgl[0:5], AF.Exp,
                                     bias=bsb[0:5, BC_G2:BC_G2 + 1], scale=1.0)
                pglR = pmainp.tile([128, TILE], fp32, tag="pmain")
                nc.tensor.matmul(pglR, lhsT=w("g2r1", 64, 128), rhs=u_g[0:64, 0:TILE],
                                 start=True, stop=True)
                eg = smallp.tile([128, TILE], fp16, tag="eg")
                nc.scalar.activation(eg, pglR, AF.Exp,
                                     bias=bsb[0:128, BC_G2R:BC_G2R + 1], scale=1.0)
                pglR4 = pmainp.tile([128, TILE], fp32, tag="pmain")
                nc.tensor.matmul(pglR4[0:29], lhsT=w("g2r2", 64, 29),
                                 rhs=u_g[0:64, 0:TILE], start=True, stop=True)
                eg4 = smallp.tile([29, TILE], fp16, tag="eg4")
                nc.scalar.activation(eg4, pglR4[0:29], AF.Exp,
                                     bias=bsb[0:29, BC_G2R4:BC_G2R4 + 1], scale=1.0)
                pd = pmainp.tile([128, TILE], fp32, tag="pmain")
                nc.tensor.matmul(pd[0:1], lhsT=w("ones5", 5, 1), rhs=t_gate,
                                 start=True, stop=True)
                rd = smallp.tile([1, TILE], fp32, tag="rd")
                nc.vector.reciprocal_approx_fast(rd, pd[0:1])
                rb29 = smallp.tile([29, TILE], fp32, tag="rb29")
                nc.gpsimd.partition_broadcast(rb29, rd, channels=29)

                # ---- blend: s_all = (pacts0 + b3') * eg covers experts 0..3
                s_all = blendp.tile([128, TILE], fp16, tag="s_all")
                nc.vector.scalar_tensor_tensor(
                    out=s_all, in0=pacts0, scalar=bsb[0:128, BC_B3:BC_B3 + 1],
                    in1=eg, op0=OP.add, op1=OP.mult)
                se4 = blendp.tile([29, TILE], fp16, tag="se4")
                nc.vector.scalar_tensor_tensor(
                    out=se4, in0=pacts1[0:29], scalar=bsb[0:29, BC_B34:BC_B34 + 1],
                    in1=eg4, op0=OP.add, op1=OP.mult)
                pbl = pmainp.tile([128, TILE], fp32, tag="pmain")
                nc.tensor.matmul(pbl[0:29], lhsT=w("msum", 128, 29), rhs=s_all,
                                 start=True, stop=False)
                nc.tensor.matmul(pbl[0:29], lhsT=w("i29", 29, 29), rhs=se4,
                                 start=False, stop=True)
                acc = blendp.tile([29, TILE], fp32, tag="acc")
                nc.vector.tensor_mul(out=acc, in0=pbl[0:29], in1=rb29)

                nc.sync.dma_start(out=out_fm[:, n0:n0 + TILE], in_=acc)

    nc.compile()
    return nc


# ----------------------------------------------------------------- host prep

def prep_inputs(x, vae_W1, vae_b1, vae_Wz, vae_bz, vae_Wv, vae_bv,
                ae_W1, ae_b1, ae_W2, ae_b2,
                gate_W1, gate_b1, gate_W2, gate_b2,
                eW1, eb1, eW2, eb2, eW3, eb3, n_rows=N_CORE, n_cores=N_CORES):
    """Returns in_maps (list of per-core dicts)."""
    x = np.asarray(x, np.float32)
    n_total = n_rows * n_cores
    assert x.shape[0] >= n_total

    xT = np.zeros((XT_ROWS, n_total), np.float16)
    xv = x[:n_total, VAE_COLS].T.astype(np.float16)  # [480, n]
    for c in range(4):
        xT[128 * c:128 * c + 120] = xv[120 * c:120 * c + 120]
    xT[504] = 1.0
    xT[512:608] = x[:n_total, ELEV_COLS].T.astype(np.float16)
    xT[608] = 1.0
    xT[640:739] = x[:n_total, OT_COLS].T.astype(np.float16)
    xT[771] = 1.0

    wpack = np.zeros((128, WCOLS), np.float32)
    bpack = np.zeros((128, NBCOLS), np.float32)
    bpack[:, BC_NEG1] = -1.0

    def put(name, idx, arr, msz=None, prow=0):
        k, m = arr.shape
        base = WOFF[name] + idx * (msz if msz is not None else m)
        wpack[prow:prow + k, base:base + m] = arr

    W1 = np.asarray(vae_W1, np.float32)  # [480, 256] rows already in vae_hist order
    for c in range(4):
        chunk = W1[120 * c:120 * c + 120]
        if c == 3:
            chunk = np.vstack([chunk, (np.asarray(vae_b1) + 1.0)[None]])
        put("w1", c, chunk, msz=256)
    # [Wv | Wz] order so the evac lands [v_pred(3) | z_H(32)] at inpB[0:35]
    Wzv = np.concatenate([vae_Wv, vae_Wz], axis=1).astype(np.float32)  # [256,35]
    put("wzv", 0, Wzv[0:128], msz=35)
    put("wzv", 1, Wzv[128:256], msz=35)
    bpack[0:35, BC_ZV] = np.concatenate([vae_bv, vae_bz]) - Wzv.sum(0)

    put("ae1", 0, np.vstack([ae_W1, (np.asarray(ae_b1) + 1.0)[None]]))
    put("ae2", 0, np.asarray(ae_W2, np.float32))
    bpack[0:32, BC_ZE] = np.asarray(ae_b2) - np.asarray(ae_W2).sum(0)

    put("g1", 0, np.vstack([gate_W1, (np.asarray(gate_b1) + 1.0)[None]]), prow=64)
    G2 = np.asarray(gate_W2, np.float32)  # [64,5]
    put("g2", 0, G2)
    bg2 = np.asarray(gate_b2) - G2.sum(0)  # [5]
    bpack[0:5, BC_G2] = bg2
    g2r1 = np.zeros((64, 128), np.float32)
    for e in range(4):
        g2r1[:, 32 * e:32 * e + 29] = G2[:, e:e + 1]
        bpack[32 * e:32 * e + 29, BC_G2R] = bg2[e]
    put("g2r1", 0, g2r1)
    g2r2 = np.repeat(G2[:, 4:5], 29, axis=1)
    put("g2r2", 0, g2r2)
    bpack[0:29, BC_G2R4] = bg2[4]

    for e in range(5):
        W1e = np.asarray(eW1[e], np.float32)  # [166,128]
        put("e1a", e, W1e[0:99], msz=128)
        e1b = np.zeros((97, 128), np.float32)
        e1b[0:35] = W1e[99:134]      # v_pred(3) + z_H(32) weight rows
        e1b[64:96] = W1e[134:166]    # z_E rows
        e1b[96] = np.asarray(eb1[e]) + 1.0
        put("e1b", e, e1b, msz=128)
        W2e = np.asarray(eW2[e], np.float32)
        c2 = np.asarray(eb2[e]) - W2e.sum(0)
        bpack[0:128, BC_C2 + e] = c2
        bpack[0:128, BC_C2P1 + e] = c2 + 1.0
        put("e2", e, W2e, msz=128)
        W3e = np.asarray(eW3[e], np.float32)
        W3p = np.zeros((128, 32), np.float32)
        W3p[:, 0:29] = W3e
        put("e3", e, W3p, msz=32)
        b3e = np.asarray(eb3[e]) - W3e.sum(0)  # [29]
        if e < 4:
            bpack[32 * e:32 * e + 29, BC_B3] = b3e
        else:
            bpack[0:29, BC_B34] = b3e
    put("ones5", 0, np.ones((5, 1), np.float32))
    msum = np.zeros((128, 29), np.float32)
    for e in range(4):
        msum[32 * e:32 * e + 29] = np.eye(29)
    put("msum", 0, msum)
    put("i29", 0, np.eye(29, dtype=np.float32))

    wpack16 = wpack.astype(np.float16)
    in_maps = []
    for c in range(n_cores):
        in_maps.append({
            "xT": np.ascontiguousarray(xT[:, c * n_rows:(c + 1) * n_rows]),
            "wpack": wpack16,
            "bpack": bpack,
        })
    return in_maps


# ----------------------------------------------------------------- entry

_NC_CACHE = {}


def _get_program(n_rows=N_CORE, num_devices=N_CORES):
    key = (n_rows, num_devices)
    if key not in _NC_CACHE:
        _NC_CACHE[key] = build_program(n_rows, num_devices)
    return _NC_CACHE[key]


def kernel(**inputs):
    from concourse.bass_utils import run_bass_kernel_spmd

    nc = _get_program()
    in_maps = prep_inputs(**inputs)
    res = run_bass_kernel_spmd(nc, in_maps, core_ids=list(range(N_CORES)))
    out = np.empty((N_FULL, 29), np.float32)
    for c in range(N_CORES):
        out[c * N_CORE:(c + 1) * N_CORE] = res.results[c]["out_fm"].T
    return out


# revision 26
# speedup vs baseline: 1.0453x; 1.0453x over previous
"""Trainium2 Bass kernel for nn_MoEAugmentedActor (moe_routing).

Strategy (pure data parallel, 8 cores, batch-sharded):
  - Host prepares a feature-major fp16 view xT of the needed x columns
    (579 of 975), with ones-rows baked in so L1 biases ride the matmul.
  - On-chip everything is feature-major: [features(part), batch(free)],
    batch tiled at 512 columns.
  - ELU(y) is computed as  elu(y)+1 = max(y+1, min(e^y, 1)):
      psum holds y+1 (bias rows are b+1), ACT does t = Exp(psum-1),
      DVE scalar_tensor_tensor does u = (t min 1) max psum  in one pass.
    The +1 shift is absorbed into the next layer's bias on the host
    (b' = b - colsum(W)).
  - Gate logits are replicated into 32-aligned 29-row blocks by an
    expanded gate-L2 matmul so per-expert softmax weights can be read
    as legal SBUF slices; softmax runs without max-subtraction
    (logits are tiny).  Blend: se_e = (pacts_e + b3'_e) * exp(gl_e)
    via one scalar_tensor_tensor per expert, summed, then multiplied
    by the broadcast reciprocal of the partition-summed exp.
  - Device writes out feature-major [29, n]; host transposes back.
"""

import os
import sys

for _p in ("/opt/trn_rl_repo", "/root/.axon_site/_ro/trn_rl_repo"):
    if os.path.isdir(_p) and _p not in sys.path:
        sys.path.insert(0, _p)

import numpy as np

# ----------------------------------------------------------------- constants
N_FULL = 131072
N_CORES = 8
N_CORE = N_FULL // N_CORES  # 16384
TILE = 512  # batch columns per tile

OBS_TERM_DIMS = (3, 3, 3, 3, 29, 29, 29, 96)
HISTORY_LEN = 5
_OFFS = [0]
for _d in OBS_TERM_DIMS[:-1]:
    _OFFS.append(_OFFS[-1] + _d * HISTORY_LEN)

# vae_hist column order: frame i in 0..4, terms 1..6, dims within term
VAE_COLS = [
    _OFFS[t] + i * OBS_TERM_DIMS[t] + j
    for i in range(HISTORY_LEN)
    for t in range(1, 7)
    for j in range(OBS_TERM_DIMS[t])
]  # 480
OT_COLS = [
    _OFFS[t] + 4 * OBS_TERM_DIMS[t] + j for t in range(7) for j in range(OBS_TERM_DIMS[t])
]  # 99
ELEV_COLS = list(range(_OFFS[7] + 4 * 96, _OFFS[7] + 5 * 96))  # 96

XT_ROWS = 784  # 6 blocks of 128 + zeros/ones tail
WCOLS = 4224


def _w_offsets():
    off = {}
    c = 0

    def take(name, n):
        nonlocal c
        off[name] = c
        c += n

    take("w1", 4 * 256)       # 4 chunks x [K,256]
    take("wzv", 2 * 35)       # 2 chunks x [128,35]  ([Wv|Wz] order)
    take("ae1", 64)           # [97,64]
    take("ae2", 32)           # [64,32]
    take("g1", 64)            # [33,64] stored at partitions 64..96
    take("g2", 5)             # [64,5]
    take("g2r1", 128)         # [64,128] replicated gate cols, experts 0..3
    take("g2r2", 29)          # [64,29]  replicated gate col, expert 4
    take("e1a", 5 * 128)      # [99,128] x5
    take("e1b", 5 * 128)      # [97,128] x5
    take("c2", 5 * 128)       # [1,128] x5
    take("e2", 5 * 128)       # [128,128] x5
    take("e3", 5 * 32)        # [128,32] x5 (padded to 32)
    take("ones5", 1)          # [5,1]
    take("msum", 29)          # [128,29] 0/1 block-sum matrix
    take("i29", 29)           # [29,29] identity
    assert c <= WCOLS, c
    return off


WOFF = _w_offsets()

# bpack columns
BC_ZV = 0      # rows 0..34:  [bv|bz]' adjusted
BC_ZE = 1      # rows 0..31:  ae_b2'
BC_G2 = 2      # rows 0..4:   gate_b2'
BC_NEG1 = 3    # all rows: -1.0
BC_G2R = 4     # rows 32e+k (e<4,k<29): gate_b2'_e   (replicated-logit bias)
BC_G2R4 = 5    # rows 0..28: gate_b2'_4
BC_B3 = 6      # rows 32e+k (e<4,k<29): b3'_e[k]
BC_B34 = 7     # rows 0..28: b3'_4
BC_C2 = 8      # cols 8..12:  expert-L2 bias c2_e (rows 0..127)
BC_C2P1 = 13   # cols 13..17: c2_e + 1
NBCOLS = 18


# ----------------------------------------------------------------- device IR

def build_program(n_rows=N_CORE, num_devices=N_CORES):
    """Build + compile the per-core Bass program. Returns nc."""
    import concourse.bass as bass
    import concourse.mybir as mybir
    from concourse import bacc
    from concourse.tile import TileContext

    fp16 = mybir.dt.float16
    fp32 = mybir.dt.float32
    AF = mybir.ActivationFunctionType
    OP = mybir.AluOpType

    n_tiles = n_rows // TILE
    assert n_rows % TILE == 0

    nc = bacc.Bacc("TRN2", target_bir_lowering=False, debug=False,
                   num_devices=num_devices)

    xT = nc.dram_tensor("xT", (XT_ROWS, n_rows), fp16, kind="ExternalInput").ap()
    wpack = nc.dram_tensor("wpack", (128, WCOLS), fp16, kind="ExternalInput").ap()
    bpack = nc.dram_tensor("bpack", (128, NBCOLS), fp32, kind="ExternalInput").ap()
    out_fm = nc.dram_tensor("out_fm", (29, n_rows), fp32, kind="ExternalOutput").ap()

    with TileContext(nc) as tc:
        with (
            tc.tile_pool(name="const", bufs=1) as constp,
            tc.tile_pool(name="xio", bufs=4) as xio,
            tc.tile_pool(name="inp", bufs=4) as inpp,
            tc.tile_pool(name="uh", bufs=6) as uhp,
            tc.tile_pool(name="small", bufs=3) as smallp,
            tc.tile_pool(name="texp", bufs=10) as texpp,
            tc.tile_pool(name="u12", bufs=8) as u12p,
            tc.tile_pool(name="blend", bufs=4) as blendp,
            tc.tile_pool(name="pe", bufs=2, space="PSUM") as pep,
            tc.tile_pool(name="pmain", bufs=3, space="PSUM") as pmainp,
            tc.tile_pool(name="pacts", bufs=1, space="PSUM") as pactsp,
        ):
            # ---- persistent constants
            wsb = constp.tile([128, WCOLS], fp16, tag="wsb")
            nc.sync.dma_start(out=wsb, in_=wpack)
            bsb = constp.tile([128, NBCOLS], fp32, tag="bsb")
            nc.sync.dma_start(out=bsb, in_=bpack)
            ones_all = constp.tile([128, TILE], fp16, tag="ones_all")
            nc.vector.memset(ones_all, 1.0)

            xT_blk = xT[0:640].rearrange("(b p) n -> p b n", p=128)  # [128, 5, n]

            def w(name, k, m, idx=0, msz=None, prow=0):
                base = WOFF[name] + idx * m
                return wsb[prow:prow + k, base:base + (msz if msz is not None else m)]

            def elu(pool, tag, psum, m, fd=TILE):
                """psum[0:m, 0:fd] holds y+1 -> u = elu(y)+1 fp16."""
                t = texpp.tile([128, 2 * TILE], fp16, tag="texp")
                nc.scalar.activation(t[0:m, 0:fd], psum[0:m, 0:fd], AF.Exp,
                                     bias=bsb[0:m, BC_NEG1:BC_NEG1 + 1], scale=1.0)
                u = pool.tile([128, 2 * TILE], fp16, tag=tag)
                nc.vector.scalar_tensor_tensor(
                    out=u[0:m, 0:fd], in0=t[0:m, 0:fd], scalar=1.0,
                    in1=psum[0:m, 0:fd], op0=OP.min, op1=OP.max)
                return u

            for it in range(n_tiles):
                n0 = it * TILE
                # ---- loads
                xsb = xio.tile([128, 5, TILE], fp16, tag="xsb")
                nc.sync.dma_start(out=xsb, in_=xT_blk[:, 0:5, n0:n0 + TILE])
                inpA = inpp.tile([128, TILE], fp16, tag="inpA")
                nc.sync.dma_start(out=inpA[0:99], in_=xT[640:739, n0:n0 + TILE])
                inpB = inpp.tile([97, TILE], fp16, tag="inpB")
                nc.sync.dma_start(out=inpB[32:64], in_=xT[739:771, n0:n0 + TILE])
                nc.sync.dma_start(out=inpB[96:97], in_=xT[771:772, n0:n0 + TILE])

                # ---- VAE L1: two 128-halves, separate psum tiles
                u_hs = []
                for half in (0, 1):
                    ph = pmainp.tile([128, TILE], fp32, tag="pmain")
                    for c in range(4):
                        k = 121 if c == 3 else 120
                        nc.tensor.matmul(
                            ph,
                            lhsT=wsb[0:k, WOFF["w1"] + c * 256 + half * 128:
                                     WOFF["w1"] + c * 256 + half * 128 + 128],
                            rhs=xsb[0:k, c, :],
                            start=(c == 0), stop=(c == 3))
                    u_hs.append(elu(uhp, "uh", ph, 128))
                u_h0, u_h1 = u_hs

                # ---- VAE L2 -> [v_pred(3) | z_H(32)] into inpB[0:35]
                pz = pmainp.tile([128, TILE], fp32, tag="pmain")
                nc.tensor.matmul(pz[0:35], lhsT=w("wzv", 128, 35, 0),
                                 rhs=u_h0[:, 0:TILE], start=True, stop=False)
                nc.tensor.matmul(pz[0:35], lhsT=w("wzv", 128, 35, 1),
                                 rhs=u_h1[:, 0:TILE], start=False, stop=True)
                nc.scalar.activation(inpB[0:35], pz[0:35], AF.Identity,
                                     bias=bsb[0:35, BC_ZV:BC_ZV + 1], scale=1.0)

                # ---- AE: ha -> z_E(32) into inpB[64:96]
                pa = pmainp.tile([128, TILE], fp32, tag="pmain")
                nc.tensor.matmul(pa[0:64], lhsT=w("ae1", 97, 64), rhs=xsb[0:97, 4, :],
                                 start=True, stop=True)
                u_a = elu(uhp, "uh", pa, 64)
                pzE = pmainp.tile([128, TILE], fp32, tag="pmain")
                nc.tensor.matmul(pzE[0:32], lhsT=w("ae2", 64, 32), rhs=u_a[0:64, 0:TILE],
                                 start=True, stop=True)
                nc.scalar.activation(inpB[64:96], pzE[0:32], AF.Identity,
                                     bias=bsb[0:32, BC_ZE:BC_ZE + 1], scale=1.0)

                # ---- experts: pairs (0,1)+(2,3) interleaved; e4 single lane
                pacts0 = pactsp.tile([128, TILE], fp32, tag="pacts")

                def l1_mms(pair):
                    pe1 = pep.tile([128, 2 * TILE], fp32, tag="pe")
                    for j, e in enumerate(pair):
                        sl = slice(j * TILE, (j + 1) * TILE)
                        nc.tensor.matmul(pe1[:, sl], lhsT=w("e1a", 99, 128, e),
                                         rhs=inpA[0:99], start=True, stop=False)
                        nc.tensor.matmul(pe1[:, sl], lhsT=w("e1b", 97, 128, e),
                                         rhs=inpB[0:97], start=False, stop=True)
                    return pe1

                def l2_mms(pair, u1):
                    pe2 = pep.tile([128, 2 * TILE], fp32, tag="pe")
                    for j, e in enumerate(pair):
                        sl = slice(j * TILE, (j + 1) * TILE)
                        nc.tensor.matmul(pe2[:, sl], lhsT=w("e2", 128, 128, e),
                                         rhs=u1[:, sl], start=True, stop=True)
                    return pe2

                def l2_elu(pair, pe2):
                    fd = len(pair) * TILE
                    t2 = texpp.tile([128, 2 * TILE], fp16, tag="texp")
                    for j, e in enumerate(pair):
                        sl = slice(j * TILE, (j + 1) * TILE)
                        nc.scalar.activation(t2[:, sl], pe2[:, sl], AF.Exp,
                                             bias=bsb[0:128, BC_C2 + e:BC_C2 + e + 1],
                                             scale=1.0)
                    s2 = texpp.tile([128, 2 * TILE], fp16, tag="s2")
                    nc.vector.tensor_scalar(out=s2[:, 0:fd], in0=t2[:, 0:fd],
                                            scalar1=1.0, scalar2=None, op0=OP.min)
                    u2 = u12p.tile([128, 2 * TILE], fp16, tag="u12")
                    for j, e in enumerate(pair):
                        sl = slice(j * TILE, (j + 1) * TILE)
                        nc.vector.scalar_tensor_tensor(
                            out=u2[:, sl], in0=pe2[:, sl],
                            scalar=bsb[0:128, BC_C2P1 + e:BC_C2P1 + e + 1],
                            in1=s2[:, sl], op0=OP.add, op1=OP.max)
                    return u2

                def l3_mms(pair, u2):
                    for j, e in enumerate(pair):
                        sl = slice(j * TILE, (j + 1) * TILE)
                        if e < 4:
                            nc.tensor.matmul(pacts0[32 * e:32 * e + 32],
                                             lhsT=w("e3", 128, 32, e), rhs=u2[:, sl],
                                             start=True, stop=True,
                                             tile_position=(0, 32 * e))
                        else:
                            pacts1 = pmainp.tile([128, TILE], fp32, tag="pmain")
                            nc.tensor.matmul(pacts1[0:29],
                                             lhsT=w("e3", 128, 32, e, msz=29),
                                             rhs=u2[:, sl], start=True, stop=True)
                            globals_pacts1[0] = pacts1
                    return globals_pacts1[0] if pair == (4,) else None

                globals_pacts1 = [None]
                pA, pB = (0, 1), (2, 3)
                pe1a = l1_mms(pA)
                pe1b = l1_mms(pB)
                u1a = elu(u12p, "u12", pe1a, 128, 2 * TILE)
                pe2a = l2_mms(pA, u1a)
                u1b = elu(u12p, "u12", pe1b, 128, 2 * TILE)
                pe2b = l2_mms(pB, u1b)
                u2a = l2_elu(pA, pe2a)
                l3_mms(pA, u2a)
                u2b = l2_elu(pB, pe2b)
                l3_mms(pB, u2b)
                # expert 4 single lane
                pe14 = pmainp.tile([128, TILE], fp32, tag="pmain")
                nc.tensor.matmul(pe14, lhsT=w("e1a", 99, 128, 4),
                                 rhs=inpA[0:99], start=True, stop=False)
                nc.tensor.matmul(pe14, lhsT=w("e1b", 97, 128, 4),
                                 rhs=inpB[0:97], start=False, stop=True)
                u14 = elu(u12p, "u12", pe14, 128)
                pe24 = pmainp.tile([128, TILE], fp32, tag="pmain")
                nc.tensor.matmul(pe24, lhsT=w("e2", 128, 128, 4),
                                 rhs=u14[:, 0:TILE], start=True, stop=True)
                u24 = l2_elu((4,), pe24)
                pacts1 = l3_mms((4,), u24)

                # ---- gate chain (emitted late: overlaps expert crunch)
                pg = pmainp.tile([128, TILE], fp32, tag="pmain")
                nc.tensor.matmul(pg[0:64], lhsT=w("g1", 33, 64, prow=64),
                                 rhs=inpB[64:97], start=True, stop=True)
                u_g = elu(uhp, "uh", pg, 64)
                pgl = pmainp.tile([128, TILE], fp32, tag="pmain")
                nc.tensor.matmul(pgl[0:5], lhsT=w("g2", 64, 5), rhs=u_g[0:64, 0:TILE],
                                 start=True, stop=True)
                t_gate = smallp.tile([5, TILE], fp16, tag="tgate")
                nc.scalar.activation(t_gate, pgl[0:5], AF.Exp,
                                     bias=bsb[0:5, BC_G2:BC_G2 + 1], scale=1.0)
                pglR = pmainp.tile([128, TILE], fp32, tag="pmain")
                nc.tensor.matmul(pglR, lhsT=w("g2r1", 64, 128), rhs=u_g[0:64, 0:TILE],
                                 start=True, stop=True)
                eg = smallp.tile([128, TILE], fp16, tag="eg")
                nc.scalar.activation(eg, pglR, AF.Exp,
                                     bias=bsb[0:128, BC_G2R:BC_G2R + 1], scale=1.0)
                pglR4 = pmainp.tile([128, TILE], fp32, tag="pmain")
                nc.tensor.matmul(pglR4[0:29], lhsT=w("g2r2", 64, 29),
                                 rhs=u_g[0:64, 0:TILE], start=True, stop=True)
                eg4 = smallp.tile([29, TILE], fp16, tag="eg4")
                nc.scalar.activation(eg4, pglR4[0:29], AF.Exp,
                                     bias=bsb[0:29, BC_G2R4:BC_G2R4 + 1], scale=1.0)
                pd = pmainp.tile([128, TILE], fp32, tag="pmain")
                nc.tensor.matmul(pd[0:1], lhsT=w("ones5", 5, 1), rhs=t_gate,
                                 start=True, stop=True)
                rd = smallp.tile([1, TILE], fp32, tag="rd")
                nc.vector.reciprocal_approx_fast(rd, pd[0:1])
                rb29 = smallp.tile([29, TILE], fp32, tag="rb29")
                nc.gpsimd.partition_broadcast(rb29, rd, channels=29)

                # ---- blend: s_all = (pacts0 + b3') * eg covers experts 0..3
                s_all = blendp.tile([128, TILE], fp16, tag="s_all")
                nc.vector.scalar_tensor_tensor(
                    out=s_all, in0=pacts0, scalar=bsb[0:128, BC_B3:BC_B3 + 1],
                    in1=eg, op0=OP.add, op1=OP.mult)
                se4 = blendp.tile([29, TILE], fp16, tag="se4")
                nc.vector.scalar_tensor_tensor(
                    out=se4, in0=pacts1[0:29], scalar=bsb[0:29, BC_B34:BC_B34 + 1],
                    in1=eg4, op0=OP.add, op1=OP.mult)
                pbl = pmainp.tile([128, TILE], fp32, tag="pmain")
                nc.tensor.matmul(pbl[0:29], lhsT=w("msum", 128, 29), rhs=s_all,
                                 start=True, stop=False)
                nc.tensor.matmul(pbl[0:29], lhsT=w("i29", 29, 29), rhs=se4,
                                 start=False, stop=True)
                acc = blendp.tile([29, TILE], fp32, tag="acc")
                nc.vector.tensor_mul(out=acc, in0=pbl[0:29], in1=rb29)

                nc.sync.dma_start(out=out_fm[:, n0:n0 + TILE], in_=acc)

    nc.compile()
    return nc


# ----------------------------------------------------------------- host prep

def prep_inputs(x, vae_W1, vae_b1, vae_Wz, vae_bz, vae_Wv, vae_bv,
                ae_W1, ae_b1, ae_W2, ae_b2,
                gate_W1, gate_b1, gate_W2, gate_b2,
                eW1, eb1, eW2, eb2, eW3, eb3, n_rows=N_CORE, n_cores=N_CORES):
    """Returns in_maps (list of per-core dicts)."""
    x = np.asarray(x, np.float32)
    n_total = n_rows * n_cores
    assert x.shape[0] >= n_total

    xT = np.zeros((XT_ROWS, n_total), np.float16)
    xv = x[:n_total, VAE_COLS].T.astype(np.float16)  # [480, n]
    for c in range(4):
        xT[128 * c:128 * c + 120] = xv[120 * c:120 * c + 120]
    xT[504] = 1.0
    xT[512:608] = x[:n_total, ELEV_COLS].T.astype(np.float16)
    xT[608] = 1.0
    xT[640:739] = x[:n_total, OT_COLS].T.astype(np.float16)
    xT[771] = 1.0

    wpack = np.zeros((128, WCOLS), np.float32)
    bpack = np.zeros((128, NBCOLS), np.float32)
    bpack[:, BC_NEG1] = -1.0

    def put(name, idx, arr, msz=None, prow=0):
        k, m = arr.shape
        base = WOFF[name] + idx * (msz if msz is not None else m)
        wpack[prow:prow + k, base:base + m] = arr

    W1 = np.asarray(vae_W1, np.float32)  # [480, 256] rows already in vae_hist order
    for c in range(4):
        chunk = W1[120 * c:120 * c + 120]
        if c == 3:
            chunk = np.vstack([chunk, (np.asarray(vae_b1) + 1.0)[None]])
        put("w1", c, chunk, msz=256)
    # [Wv | Wz] order so the evac lands [v_pred(3) | z_H(32)] at inpB[0:35]
    Wzv = np.concatenate([vae_Wv, vae_Wz], axis=1).astype(np.float32)  # [256,35]
    put("wzv", 0, Wzv[0:128], msz=35)
    put("wzv", 1, Wzv[128:256], msz=35)
    bpack[0:35, BC_ZV] = np.concatenate([vae_bv, vae_bz]) - Wzv.sum(0)

    put("ae1", 0, np.vstack([ae_W1, (np.asarray(ae_b1) + 1.0)[None]]))
    put("ae2", 0, np.asarray(ae_W2, np.float32))
    bpack[0:32, BC_ZE] = np.asarray(ae_b2) - np.asarray(ae_W2).sum(0)

    put("g1", 0, np.vstack([gate_W1, (np.asarray(gate_b1) + 1.0)[None]]), prow=64)
    G2 = np.asarray(gate_W2, np.float32)  # [64,5]
    put("g2", 0, G2)
    bg2 = np.asarray(gate_b2) - G2.sum(0)  # [5]
    bpack[0:5, BC_G2] = bg2
    g2r1 = np.zeros((64, 128), np.float32)
    for e in range(4):
        g2r1[:, 32 * e:32 * e + 29] = G2[:, e:e + 1]
        bpack[32 * e:32 * e + 29, BC_G2R] = bg2[e]
    put("g2r1", 0, g2r1)
    g2r2 = np.repeat(G2[:, 4:5], 29, axis=1)
    put("g2r2", 0, g2r2)
    bpack[0:29, BC_G2R4] = bg2[4]

    for e in range(5):
        W1e = np.asarray(eW1[e], np.float32)  # [166,128]
        put("e1a", e, W1e[0:99], msz=128)
        e1b = np.zeros((97, 128), np.float32)
        e1b[0:35] = W1e[99:134]      # v_pred(3) + z_H(32) weight rows
        e1b[64:96] = W1e[134:166]    # z_E rows
        e1b[96] = np.asarray(eb1[e]) + 1.0
        put("e1b", e, e1b, msz=128)
        W2e = np.asarray(eW2[e], np.float32)
        c2 = np.asarray(eb2[e]) - W2e.sum(0)
        bpack[0:128, BC_C2 + e] = c2
        bpack[0:128, BC_C2P1 + e] = c2 + 1.0
        put("e2", e, W2e, msz=128)
        W3e = np.asarray(eW3[e], np.float32)
        W3p = np.zeros((128, 32), np.float32)
        W3p[:, 0:29] = W3e
        put("e3", e, W3p, msz=32)
        b3e = np.asarray(eb3[e]) - W3e.sum(0)  # [29]
        if e < 4:
            bpack[32 * e:32 * e + 29, BC_B3] = b3e
        else:
            bpack[0:29, BC_B34] = b3e
    put("ones5", 0, np.ones((5, 1), np.float32))
    msum = np.zeros((128, 29), np.float32)
    for e in range(4):
        msum[32 * e:32 * e + 29] = np.eye(29)
    put("msum", 0, msum)
    put("i29", 0, np.eye(29, dtype=np.float32))

    wpack16 = wpack.astype(np.float16)
    in_maps = []
    for c in range(n_cores):
        in_maps.append({
            "xT": np.ascontiguousarray(xT[:, c * n_rows:(c + 1) * n_rows]),
            "wpack": wpack16,
            "bpack": bpack,
        })
    return in_maps


# ----------------------------------------------------------------- entry

_NC_CACHE = {}


def _get_program(n_rows=N_CORE, num_devices=N_CORES):
    key = (n_rows, num_devices)
    if key not in _NC_CACHE:
        _NC_CACHE[key] = build_program(n_rows, num_devices)
    return _NC_CACHE[key]


def kernel(**inputs):
    from concourse.bass_utils import run_bass_kernel_spmd

    nc = _get_program()
    in_maps = prep_inputs(**inputs)
    res = run_bass_kernel_spmd(nc, in_maps, core_ids=list(range(N_CORES)))
    out = np.empty((N_FULL, 29), np.float32)
    for c in range(N_CORES):
        out[c * N_CORE:(c + 1) * N_CORE] = res.results[c]["out_fm"].T
    return out


# revision 28
# speedup vs baseline: 1.0455x; 1.0001x over previous
"""Trainium2 Bass kernel for nn_MoEAugmentedActor (moe_routing).

Strategy (pure data parallel, 8 cores, batch-sharded):
  - Host prepares a feature-major fp16 view xT of the needed x columns
    (579 of 975), with ones-rows baked in so L1 biases ride the matmul.
  - On-chip everything is feature-major: [features(part), batch(free)],
    batch tiled at 512 columns.
  - ELU(y) is computed as  elu(y)+1 = max(y+1, min(e^y, 1)):
      psum holds y+1 (bias rows are b+1), ACT does t = Exp(psum-1),
      DVE scalar_tensor_tensor does u = (t min 1) max psum  in one pass.
    The +1 shift is absorbed into the next layer's bias on the host
    (b' = b - colsum(W)).
  - Gate logits are replicated into 32-aligned 29-row blocks by an
    expanded gate-L2 matmul so per-expert softmax weights can be read
    as legal SBUF slices; softmax runs without max-subtraction
    (logits are tiny).  Blend: se_e = (pacts_e + b3'_e) * exp(gl_e)
    via one scalar_tensor_tensor per expert, summed, then multiplied
    by the broadcast reciprocal of the partition-summed exp.
  - Device writes out feature-major [29, n]; host transposes back.
"""

import os
import sys

for _p in ("/opt/trn_rl_repo", "/root/.axon_site/_ro/trn_rl_repo"):
    if os.path.isdir(_p) and _p not in sys.path:
        sys.path.insert(0, _p)

import numpy as np

# ----------------------------------------------------------------- constants
N_FULL = 131072
N_CORES = 8
N_CORE = N_FULL // N_CORES  # 16384
TILE = 512  # batch columns per tile

OBS_TERM_DIMS = (3, 3, 3, 3, 29, 29, 29, 96)
HISTORY_LEN = 5
_OFFS = [0]
for _d in OBS_TERM_DIMS[:-1]:
    _OFFS.append(_OFFS[-1] + _d * HISTORY_LEN)

# vae_hist column order: frame i in 0..4, terms 1..6, dims within term
VAE_COLS = [
    _OFFS[t] + i * OBS_TERM_DIMS[t] + j
    for i in range(HISTORY_LEN)
    for t in range(1, 7)
    for j in range(OBS_TERM_DIMS[t])
]  # 480
OT_COLS = [
    _OFFS[t] + 4 * OBS_TERM_DIMS[t] + j for t in range(7) for j in range(OBS_TERM_DIMS[t])
]  # 99
ELEV_COLS = list(range(_OFFS[7] + 4 * 96, _OFFS[7] + 5 * 96))  # 96

XT_ROWS = 784  # 6 blocks of 128 + zeros/ones tail
WCOLS = 4224


def _w_offsets():
    off = {}
    c = 0

    def take(name, n):
        nonlocal c
        off[name] = c
        c += n

    take("w1", 4 * 256)       # 4 chunks x [K,256]
    take("wzv", 2 * 35)       # 2 chunks x [128,35]  ([Wv|Wz] order)
    take("ae1", 64)           # [97,64]
    take("ae2", 32)           # [64,32]
    take("g1", 64)            # [33,64] stored at partitions 64..96
    take("g2", 5)             # [64,5]
    take("g2r1", 128)         # [64,128] replicated gate cols, experts 0..3
    take("g2r2", 29)          # [64,29]  replicated gate col, expert 4
    take("e1a", 5 * 128)      # [99,128] x5
    take("e1b", 5 * 128)      # [97,128] x5
    take("c2", 5 * 128)       # [1,128] x5
    take("e2", 5 * 128)       # [128,128] x5
    take("e3", 5 * 32)        # [128,32] x5 (padded to 32)
    take("ones5", 1)          # [5,1]
    take("msum", 29)          # [128,29] 0/1 block-sum matrix
    take("i29", 29)           # [29,29] identity
    assert c <= WCOLS, c
    return off


WOFF = _w_offsets()

# bpack columns
BC_ZV = 0      # rows 0..34:  [bv|bz]' adjusted
BC_ZE = 1      # rows 0..31:  ae_b2'
BC_G2 = 2      # rows 0..4:   gate_b2'
BC_NEG1 = 3    # all rows: -1.0
BC_G2R = 4     # rows 32e+k (e<4,k<29): gate_b2'_e   (replicated-logit bias)
BC_G2R4 = 5    # rows 0..28: gate_b2'_4
BC_B3 = 6      # rows 32e+k (e<4,k<29): b3'_e[k]
BC_B34 = 7     # rows 0..28: b3'_4
BC_C2 = 8      # cols 8..12:  expert-L2 bias c2_e (rows 0..127)
BC_C2P1 = 13   # cols 13..17: c2_e + 1
NBCOLS = 18


# ----------------------------------------------------------------- device IR

def build_program(n_rows=N_CORE, num_devices=N_CORES):
    """Build + compile the per-core Bass program. Returns nc."""
    import concourse.bass as bass
    import concourse.mybir as mybir
    from concourse import bacc
    from concourse.tile import TileContext

    fp16 = mybir.dt.float16
    fp32 = mybir.dt.float32
    AF = mybir.ActivationFunctionType
    OP = mybir.AluOpType

    n_tiles = n_rows // TILE
    assert n_rows % TILE == 0

    nc = bacc.Bacc("TRN2", target_bir_lowering=False, debug=False,
                   num_devices=num_devices)

    xT = nc.dram_tensor("xT", (XT_ROWS, n_rows), fp16, kind="ExternalInput").ap()
    wpack = nc.dram_tensor("wpack", (128, WCOLS), fp16, kind="ExternalInput").ap()
    bpack = nc.dram_tensor("bpack", (128, NBCOLS), fp32, kind="ExternalInput").ap()
    out_fm = nc.dram_tensor("out_fm", (29, n_rows), fp32, kind="ExternalOutput").ap()

    with TileContext(nc) as tc:
        with (
            tc.tile_pool(name="const", bufs=1) as constp,
            tc.tile_pool(name="xio", bufs=4) as xio,
            tc.tile_pool(name="inp", bufs=4) as inpp,
            tc.tile_pool(name="uh", bufs=6) as uhp,
            tc.tile_pool(name="small", bufs=3) as smallp,
            tc.tile_pool(name="texp", bufs=10) as texpp,
            tc.tile_pool(name="u12", bufs=8) as u12p,
            tc.tile_pool(name="blend", bufs=4) as blendp,
            tc.tile_pool(name="pe", bufs=2, space="PSUM") as pep,
            tc.tile_pool(name="pmain", bufs=3, space="PSUM") as pmainp,
            tc.tile_pool(name="pacts", bufs=1, space="PSUM") as pactsp,
        ):
            # ---- persistent constants
            wsb = constp.tile([128, WCOLS], fp16, tag="wsb")
            nc.sync.dma_start(out=wsb, in_=wpack)
            bsb = constp.tile([128, NBCOLS], fp32, tag="bsb")
            nc.sync.dma_start(out=bsb, in_=bpack)
            ones_all = constp.tile([128, TILE], fp16, tag="ones_all")
            nc.vector.memset(ones_all, 1.0)

            xT_blk = xT[0:640].rearrange("(b p) n -> p b n", p=128)  # [128, 5, n]

            def w(name, k, m, idx=0, msz=None, prow=0):
                base = WOFF[name] + idx * m
                return wsb[prow:prow + k, base:base + (msz if msz is not None else m)]

            def elu(pool, tag, psum, m, fd=TILE):
                """psum[0:m, 0:fd] holds y+1 -> u = elu(y)+1 fp16."""
                t = texpp.tile([128, 2 * TILE], fp16, tag="texp")
                nc.scalar.activation(t[0:m, 0:fd], psum[0:m, 0:fd], AF.Exp,
                                     bias=bsb[0:m, BC_NEG1:BC_NEG1 + 1], scale=1.0)
                u = pool.tile([128, 2 * TILE], fp16, tag=tag)
                nc.vector.scalar_tensor_tensor(
                    out=u[0:m, 0:fd], in0=t[0:m, 0:fd], scalar=1.0,
                    in1=psum[0:m, 0:fd], op0=OP.min, op1=OP.max)
                return u

            for it in range(n_tiles):
                n0 = it * TILE
                # ---- loads
                xsb = xio.tile([128, 5, TILE], fp16, tag="xsb")
                nc.sync.dma_start(out=xsb, in_=xT_blk[:, 0:5, n0:n0 + TILE])
                inpA = inpp.tile([128, TILE], fp16, tag="inpA")
                nc.sync.dma_start(out=inpA[0:99], in_=xT[640:739, n0:n0 + TILE])
                inpB = inpp.tile([97, TILE], fp16, tag="inpB")
                nc.sync.dma_start(out=inpB[32:64], in_=xT[739:771, n0:n0 + TILE])
                nc.sync.dma_start(out=inpB[96:97], in_=xT[771:772, n0:n0 + TILE])

                # early expert-L1 A-chunks: only need inpA (o_t DMA)
                pe1a = pep.tile([128, 2 * TILE], fp32, tag="pe")
                pe1b = pep.tile([128, 2 * TILE], fp32, tag="pe")
                for pe1_, pair_ in ((pe1a, (0, 1)), (pe1b, (2, 3))):
                    for j_, e_ in enumerate(pair_):
                        nc.tensor.matmul(pe1_[:, j_ * TILE:(j_ + 1) * TILE],
                                         lhsT=w("e1a", 99, 128, e_),
                                         rhs=inpA[0:99], start=True, stop=False)

                # ---- VAE L1: two 128-halves, separate psum tiles
                u_hs = []
                for half in (0, 1):
                    ph = pmainp.tile([128, TILE], fp32, tag="pmain")
                    for c in range(4):
                        k = 121 if c == 3 else 120
                        nc.tensor.matmul(
                            ph,
                            lhsT=wsb[0:k, WOFF["w1"] + c * 256 + half * 128:
                                     WOFF["w1"] + c * 256 + half * 128 + 128],
                            rhs=xsb[0:k, c, :],
                            start=(c == 0), stop=(c == 3))
                    u_hs.append(elu(uhp, "uh", ph, 128))
                u_h0, u_h1 = u_hs

                # ---- VAE L2 -> [v_pred(3) | z_H(32)] into inpB[0:35]
                pz = pmainp.tile([128, TILE], fp32, tag="pmain")
                nc.tensor.matmul(pz[0:35], lhsT=w("wzv", 128, 35, 0),
                                 rhs=u_h0[:, 0:TILE], start=True, stop=False)
                nc.tensor.matmul(pz[0:35], lhsT=w("wzv", 128, 35, 1),
                                 rhs=u_h1[:, 0:TILE], start=False, stop=True)
                nc.scalar.activation(inpB[0:35], pz[0:35], AF.Identity,
                                     bias=bsb[0:35, BC_ZV:BC_ZV + 1], scale=1.0)

                # ---- AE: ha -> z_E(32) into inpB[64:96]
                pa = pmainp.tile([128, TILE], fp32, tag="pmain")
                nc.tensor.matmul(pa[0:64], lhsT=w("ae1", 97, 64), rhs=xsb[0:97, 4, :],
                                 start=True, stop=True)
                u_a = elu(uhp, "uh", pa, 64)
                pzE = pmainp.tile([128, TILE], fp32, tag="pmain")
                nc.tensor.matmul(pzE[0:32], lhsT=w("ae2", 64, 32), rhs=u_a[0:64, 0:TILE],
                                 start=True, stop=True)
                nc.scalar.activation(inpB[64:96], pzE[0:32], AF.Identity,
                                     bias=bsb[0:32, BC_ZE:BC_ZE + 1], scale=1.0)

                # ---- experts: pairs (0,1)+(2,3) interleaved; e4 single lane
                pacts0 = pactsp.tile([128, TILE], fp32, tag="pacts")

                def l1b_mms(pair, pe1):
                    for j, e in enumerate(pair):
                        sl = slice(j * TILE, (j + 1) * TILE)
                        nc.tensor.matmul(pe1[:, sl], lhsT=w("e1b", 97, 128, e),
                                         rhs=inpB[0:97], start=False, stop=True)

                def l2_mms(pair, u1):
                    pe2 = pep.tile([128, 2 * TILE], fp32, tag="pe")
                    for j, e in enumerate(pair):
                        sl = slice(j * TILE, (j + 1) * TILE)
                        nc.tensor.matmul(pe2[:, sl], lhsT=w("e2", 128, 128, e),
                                         rhs=u1[:, sl], start=True, stop=True)
                    return pe2

                def l2_elu(pair, pe2):
                    fd = len(pair) * TILE
                    t2 = texpp.tile([128, 2 * TILE], fp16, tag="texp")
                    for j, e in enumerate(pair):
                        sl = slice(j * TILE, (j + 1) * TILE)
                        nc.scalar.activation(t2[:, sl], pe2[:, sl], AF.Exp,
                                             bias=bsb[0:128, BC_C2 + e:BC_C2 + e + 1],
                                             scale=1.0)
                    s2 = texpp.tile([128, 2 * TILE], fp16, tag="s2")
                    nc.vector.tensor_scalar(out=s2[:, 0:fd], in0=t2[:, 0:fd],
                                            scalar1=1.0, scalar2=None, op0=OP.min)
                    u2 = u12p.tile([128, 2 * TILE], fp16, tag="u12")
                    for j, e in enumerate(pair):
                        sl = slice(j * TILE, (j + 1) * TILE)
                        nc.vector.scalar_tensor_tensor(
                            out=u2[:, sl], in0=pe2[:, sl],
                            scalar=bsb[0:128, BC_C2P1 + e:BC_C2P1 + e + 1],
                            in1=s2[:, sl], op0=OP.add, op1=OP.max)
                    return u2

                def l3_mms(pair, u2):
                    for j, e in enumerate(pair):
                        sl = slice(j * TILE, (j + 1) * TILE)
                        if e < 4:
                            nc.tensor.matmul(pacts0[32 * e:32 * e + 32],
                                             lhsT=w("e3", 128, 32, e), rhs=u2[:, sl],
                                             start=True, stop=True,
                                             tile_position=(0, 32 * e))
                        else:
                            pacts1 = pmainp.tile([128, TILE], fp32, tag="pmain")
                            nc.tensor.matmul(pacts1[0:29],
                                             lhsT=w("e3", 128, 32, e, msz=29),
                                             rhs=u2[:, sl], start=True, stop=True)
                            globals_pacts1[0] = pacts1
                    return globals_pacts1[0] if pair == (4,) else None

                globals_pacts1 = [None]
                pA, pB = (0, 1), (2, 3)
                l1b_mms(pA, pe1a)
                l1b_mms(pB, pe1b)
                u1a = elu(u12p, "u12", pe1a, 128, 2 * TILE)
                pe2a = l2_mms(pA, u1a)
                u1b = elu(u12p, "u12", pe1b, 128, 2 * TILE)
                pe2b = l2_mms(pB, u1b)
                u2a = l2_elu(pA, pe2a)
                l3_mms(pA, u2a)
                u2b = l2_elu(pB, pe2b)
                l3_mms(pB, u2b)
                # expert 4 single lane
                pe14 = pmainp.tile([128, TILE], fp32, tag="pmain")
                nc.tensor.matmul(pe14, lhsT=w("e1a", 99, 128, 4),
                                 rhs=inpA[0:99], start=True, stop=False)
                nc.tensor.matmul(pe14, lhsT=w("e1b", 97, 128, 4),
                                 rhs=inpB[0:97], start=False, stop=True)
                u14 = elu(u12p, "u12", pe14, 128)
                pe24 = pmainp.tile([128, TILE], fp32, tag="pmain")
                nc.tensor.matmul(pe24, lhsT=w("e2", 128, 128, 4),
                                 rhs=u14[:, 0:TILE], start=True, stop=True)
                u24 = l2_elu((4,), pe24)
                pacts1 = l3_mms((4,), u24)

                # ---- gate chain (emitted late: overlaps expert crunch)
                pg = pmainp.tile([128, TILE], fp32, tag="pmain")
                nc.tensor.matmul(pg[0:64], lhsT=w("g1", 33, 64, prow=64),
                                 rhs=inpB[64:97], start=True, stop=True)
                u_g = elu(uhp, "uh", pg, 64)
                pgl = pmainp.tile([128, TILE], fp32, tag="pmain")
                nc.tensor.matmul(pgl[0:5], lhsT=w("g2", 64, 5), rhs=u_g[0:64, 0:TILE],
                                 start=True, stop=True)
                t_gate = smallp.tile([5, TILE], fp16, tag="tgate")
                nc.scalar.activation(t_gate, pgl[0:5], AF.Exp,
                                     bias=bsb[0:5, BC_G2:BC_G2 + 1], scale=1.0)
                pglR = pmainp.tile([128, TILE], fp32, tag="pmain")
                nc.tensor.matmul(pglR, lhsT=w("g2r1", 64, 128), rhs=u_g[0:64, 0:TILE],
                                 start=True, stop=True)
                eg = smallp.tile([128, TILE], fp16, tag="eg")
                nc.scalar.activation(eg, pglR, AF.Exp,
                                     bias=bsb[0:128, BC_G2R:BC_G2R + 1], scale=1.0)
                pglR4 = pmainp.tile([128, TILE], fp32, tag="pmain")
                nc.tensor.matmul(pglR4[0:29], lhsT=w("g2r2", 64, 29),
                                 rhs=u_g[0:64, 0:TILE], start=True, stop=True)
                eg4 = smallp.tile([29, TILE], fp16, tag="eg4")
                nc.scalar.activation(eg4, pglR4[0:29], AF.Exp,
                                     bias=bsb[0:29, BC_G2R4:BC_G2R4 + 1], scale=1.0)
                pd = pmainp.tile([128, TILE], fp32, tag="pmain")
                nc.tensor.matmul(pd[0:1], lhsT=w("ones5", 5, 1), rhs=t_gate,
                                 start=True, stop=True)
                rd = smallp.tile([1, TILE], fp32, tag="rd")
                nc.vector.reciprocal_approx_fast(rd, pd[0:1])
                rb29 = smallp.tile([29, TILE], fp32, tag="rb29")
                nc.gpsimd.partition_broadcast(rb29, rd, channels=29)

                # ---- blend: s_all = (pacts0 + b3') * eg covers experts 0..3
                s_all = blendp.tile([128, TILE], fp16, tag="s_all")
                nc.vector.scalar_tensor_tensor(
                    out=s_all, in0=pacts0, scalar=bsb[0:128, BC_B3:BC_B3 + 1],
                    in1=eg, op0=OP.add, op1=OP.mult)
                se4 = blendp.tile([29, TILE], fp16, tag="se4")
                nc.vector.scalar_tensor_tensor(
                    out=se4, in0=pacts1[0:29], scalar=bsb[0:29, BC_B34:BC_B34 + 1],
                    in1=eg4, op0=OP.add, op1=OP.mult)
                pbl = pmainp.tile([128, TILE], fp32, tag="pmain")
                nc.tensor.matmul(pbl[0:29], lhsT=w("msum", 128, 29), rhs=s_all,
                                 start=True, stop=False)
                nc.tensor.matmul(pbl[0:29], lhsT=w("i29", 29, 29), rhs=se4,
                                 start=False, stop=True)
                acc = blendp.tile([29, TILE], fp32, tag="acc")
                nc.vector.tensor_mul(out=acc, in0=pbl[0:29], in1=rb29)

                nc.sync.dma_start(out=out_fm[:, n0:n0 + TILE], in_=acc)

    nc.compile()
    return nc


# ----------------------------------------------------------------- host prep

def prep_inputs(x, vae_W1, vae_b1, vae_Wz, vae_bz, vae_Wv, vae_bv,
                ae_W1, ae_b1, ae_W2, ae_b2,
                gate_W1, gate_b1, gate_W2, gate_b2,
                eW1, eb1, eW2, eb2, eW3, eb3, n_rows=N_CORE, n_cores=N_CORES):
    """Returns in_maps (list of per-core dicts)."""
    x = np.asarray(x, np.float32)
    n_total = n_rows * n_cores
    assert x.shape[0] >= n_total

    xT = np.zeros((XT_ROWS, n_total), np.float16)
    xv = x[:n_total, VAE_COLS].T.astype(np.float16)  # [480, n]
    for c in range(4):
        xT[128 * c:128 * c + 120] = xv[120 * c:120 * c + 120]
    xT[504] = 1.0
    xT[512:608] = x[:n_total, ELEV_COLS].T.astype(np.float16)
    xT[608] = 1.0
    xT[640:739] = x[:n_total, OT_COLS].T.astype(np.float16)
    xT[771] = 1.0

    wpack = np.zeros((128, WCOLS), np.float32)
    bpack = np.zeros((128, NBCOLS), np.float32)
    bpack[:, BC_NEG1] = -1.0

    def put(name, idx, arr, msz=None, prow=0):
        k, m = arr.shape
        base = WOFF[name] + idx * (msz if msz is not None else m)
        wpack[prow:prow + k, base:base + m] = arr

    W1 = np.asarray(vae_W1, np.float32)  # [480, 256] rows already in vae_hist order
    for c in range(4):
        chunk = W1[120 * c:120 * c + 120]
        if c == 3:
            chunk = np.vstack([chunk, (np.asarray(vae_b1) + 1.0)[None]])
        put("w1", c, chunk, msz=256)
    # [Wv | Wz] order so the evac lands [v_pred(3) | z_H(32)] at inpB[0:35]
    Wzv = np.concatenate([vae_Wv, vae_Wz], axis=1).astype(np.float32)  # [256,35]
    put("wzv", 0, Wzv[0:128], msz=35)
    put("wzv", 1, Wzv[128:256], msz=35)
    bpack[0:35, BC_ZV] = np.concatenate([vae_bv, vae_bz]) - Wzv.sum(0)

    put("ae1", 0, np.vstack([ae_W1, (np.asarray(ae_b1) + 1.0)[None]]))
    put("ae2", 0, np.asarray(ae_W2, np.float32))
    bpack[0:32, BC_ZE] = np.asarray(ae_b2) - np.asarray(ae_W2).sum(0)

    put("g1", 0, np.vstack([gate_W1, (np.asarray(gate_b1) + 1.0)[None]]), prow=64)
    G2 = np.asarray(gate_W2, np.float32)  # [64,5]
    put("g2", 0, G2)
    bg2 = np.asarray(gate_b2) - G2.sum(0)  # [5]
    bpack[0:5, BC_G2] = bg2
    g2r1 = np.zeros((64, 128), np.float32)
    for e in range(4):
        g2r1[:, 32 * e:32 * e + 29] = G2[:, e:e + 1]
        bpack[32 * e:32 * e + 29, BC_G2R] = bg2[e]
    put("g2r1", 0, g2r1)
    g2r2 = np.repeat(G2[:, 4:5], 29, axis=1)
    put("g2r2", 0, g2r2)
    bpack[0:29, BC_G2R4] = bg2[4]

    for e in range(5):
        W1e = np.asarray(eW1[e], np.float32)  # [166,128]
        put("e1a", e, W1e[0:99], msz=128)
        e1b = np.zeros((97, 128), np.float32)
        e1b[0:35] = W1e[99:134]      # v_pred(3) + z_H(32) weight rows
        e1b[64:96] = W1e[134:166]    # z_E rows
        e1b[96] = np.asarray(eb1[e]) + 1.0
        put("e1b", e, e1b, msz=128)
        W2e = np.asarray(eW2[e], np.float32)
        c2 = np.asarray(eb2[e]) - W2e.sum(0)
        bpack[0:128, BC_C2 + e] = c2
        bpack[0:128, BC_C2P1 + e] = c2 + 1.0
        put("e2", e, W2e, msz=128)
        W3e = np.asarray(eW3[e], np.float32)
        W3p = np.zeros((128, 32), np.float32)
        W3p[:, 0:29] = W3e
        put("e3", e, W3p, msz=32)
        b3e = np.asarray(eb3[e]) - W3e.sum(0)  # [29]
        if e < 4:
            bpack[32 * e:32 * e + 29, BC_B3] = b3e
        else:
            bpack[0:29, BC_B34] = b3e
    put("ones5", 0, np.ones((5, 1), np.float32))
    msum = np.zeros((128, 29), np.float32)
    for e in range(4):
        msum[32 * e:32 * e + 29] = np.eye(29)
    put("msum", 0, msum)
    put("i29", 0, np.eye(29, dtype=np.float32))

    wpack16 = wpack.astype(np.float16)
    in_maps = []
    for c in range(n_cores):
        in_maps.append({
            "xT": np.ascontiguousarray(xT[:, c * n_rows:(c + 1) * n_rows]),
            "wpack": wpack16,
            "bpack": bpack,
        })
    return in_maps


# ----------------------------------------------------------------- entry

_NC_CACHE = {}


def _get_program(n_rows=N_CORE, num_devices=N_CORES):
    key = (n_rows, num_devices)
    if key not in _NC_CACHE:
        _NC_CACHE[key] = build_program(n_rows, num_devices)
    return _NC_CACHE[key]


def kernel(**inputs):
    from concourse.bass_utils import run_bass_kernel_spmd

    nc = _get_program()
    in_maps = prep_inputs(**inputs)
    res = run_bass_kernel_spmd(nc, in_maps, core_ids=list(range(N_CORES)))
    out = np.empty((N_FULL, 29), np.float32)
    for c in range(N_CORES):
        out[c * N_CORE:(c + 1) * N_CORE] = res.results[c]["out_fm"].T
    return out
